# revision 42
# baseline (speedup 1.0000x reference)
"""Trainium2 Bass kernel for AssetSimilarityNetwork (pairwise-MLP similarity).

Computation (reference):
    proj = af @ Wp.T + bp                      # [N, 32]
    pa   = proj @ Wa.T  (Wa = W1[:, :32])      # [N, 32]
    pb   = proj @ Wb.T  (Wb = W1[:, 32:])      # [N, 32]
    h1   = relu(pa_i + pb_j + b1)              # per pair, 32
    h2   = relu(W2 @ h1 + b2)                  # per pair, 16
    sim  = sigmoid(w3 . h2 + b3)               # [N, N], diag forced to 1

Distribution: row-shard the N^2 grid over 8 NeuronCores (256 rows each).
Each core computes the full (tiny) projection locally; no collectives.

Per-core dataflow (bf16 compute, f32 accumulate):
  A : h1 = relu(pbT_rep4 + c_col)        DVE tensor_scalar (add, max) 4x bf16
  L2: h2 pre-act via 8 tile_position-packed matmuls (K=64, M=32) -> PSUM f32
  B : h2r = relu(psum + b2)              ACT/DVE split, writes bf16 SBUF
  L3: logits via j-dense matmuls: lhsT = h2r 128-col slice (stationary),
      rhs = W3 block-diag [128, 8]  -> PSUM [128 j, 8 i] (dense partitions)
  C : sigmoid(psum + b3) -> SBUF f32 -> DMA to HBM
"""

import sys
import types

import ml_dtypes
import numpy as np

# ---------------------------------------------------------------- axon shim
sys.path.insert(0, "/root/.axon_site")
import antenv  # noqa: E402

if "antenv.axon_hooks" not in sys.modules:
    from trn_agent_boot.trn_boot import _ntff_profile_via_ctypes

    _mod = types.ModuleType("antenv.axon_hooks")
    try:
        _hook = _ntff_profile_via_ctypes("/opt/axon/libaxon_pjrt.so")
    except Exception:
        _hook = None
    _mod.get_axon_ntff_profile_hook = lambda: _hook
    _mod.set_axon_ntff_profile_hook = lambda h: None
    sys.modules["antenv.axon_hooks"] = _mod
    antenv.axon_hooks = _mod

import concourse.bass as bass  # noqa: E402
import concourse.tile as tile  # noqa: E402
from concourse import bacc, mybir  # noqa: E402
import concourse.bass_utils as bass_utils  # noqa: E402

bass_utils.upload_artifacts = lambda tmpdir: "(skipped)"
from concourse.bass_utils import run_bass_kernel_spmd  # noqa: E402

bf16 = mybir.dt.bfloat16
f32 = mybir.dt.float32
Alu = mybir.AluOpType
Act = mybir.ActivationFunctionType

N = 2048
FEAT = 64
NCORES = 8
ROWS = N // NCORES        # 256 i-rows per core
NST = ROWS // 16          # 16 super-tiles of 16 i's
NJT = N // 512            # 4 j-tiles of 512

_CACHE = {}


def _build_program():
    nc = bacc.Bacc()

    dp = nc.declare_dram_parameter
    afT = dp("afT", [FEAT + 1, N], bf16, isOutput=False)      # af.T ; ones (bf16)
    afs = dp("afs", [FEAT + 1, ROWS], bf16, isOutput=False)   # slab af.T ; ones (bf16)
    WpT = dp("WpT", [FEAT + 1, 32], bf16, isOutput=False)      # Wp.T ; bp
    WaTb = dp("WaTb", [33, 128], bf16, isOutput=False)         # (Wa.T;b1) tiled 4x cols
    WbT4 = dp("WbT4", [32, 128], bf16, isOutput=False)         # Wb.T tiled 4x cols
    W2bd = dp("W2bd", [128, 32], bf16, isOutput=False)         # L2 block-diag (2x stacked)
    W3bd = dp("W3bd", [128, 8], bf16, isOutput=False)          # L3 block-diag
    b2col = dp("b2col", [128, 1], f32, isOutput=False)
    b3col = dp("b3col", [128, 1], f32, isOutput=False)
    out_d = dp("outT", [N, ROWS], f32, isOutput=True)  # transposed: [j, i]

    with tile.TileContext(nc, num_cores=NCORES) as tc:
        _build_body(nc, tc, afT, afs, WpT, WaTb, WbT4, W2bd, W3bd, b2col, b3col, out_d)
    nc.compile()
    return nc


def _build_body(nc, tc, afT, afs, WpT, WaTb, WbT4, W2bd, W3bd, b2col, b3col, out_d):
    from contextlib import ExitStack

    ctx = ExitStack()
    const = ctx.enter_context(tc.tile_pool(name="const", bufs=1))
    h1p = ctx.enter_context(tc.tile_pool(name="h1p", bufs=2))
    h2p = ctx.enter_context(tc.tile_pool(name="h2p", bufs=4))
    sigp = ctx.enter_context(tc.tile_pool(name="sigp", bufs=2))
    pre_ctx = ExitStack()
    psA = pre_ctx.enter_context(tc.tile_pool(name="psA", bufs=2, space="PSUM"))

    # ---------------- load + cast constants ----------------
    # Small tensors first (they gate the pa path); afT is chunked so the
    # cast + projT pipeline overlaps its DMA. Warmup matmuls are issued
    # between loads to flip the PE clock gate without delaying projT much.
    def load_direct(name, param, p, fdim):
        tb = const.tile([p, fdim], bf16, name=f"{name}_b")
        nc.scalar.dma_start(tb[:], param[:])
        return tb

    afs_bf = load_direct("afs", afs, FEAT + 1, ROWS)
    WpT_bf = load_direct("WpT", WpT, FEAT + 1, 32)
    WaTb_bf = load_direct("WaTb", WaTb, 33, 128)
    WbT4_bf = load_direct("WbT4", WbT4, 32, 128)
    W2bd_bf = load_direct("W2bd", W2bd, 128, 32)
    W3bd_bf = load_direct("W3bd", W3bd, 128, 8)
    b2c = const.tile([128, 1], f32)
    nc.scalar.dma_start(b2c[:], b2col[:])
    b3c = const.tile([128, 1], f32)
    nc.scalar.dma_start(b3c[:], b3col[:])

    warm_sb = const.tile([128, 512], bf16)
    nc.vector.memset(warm_sb[:], 0.0)
    warm_ps = psA.tile([128, 512], f32, name="warm", tag="warm")
    for w in range(3):
        nc.tensor.matmul(warm_ps[:], warm_sb[:, :128], warm_sb[:],
                         start=True, stop=True)

    afT_bf = const.tile([FEAT + 1, N], bf16)
    for c4 in range(4):
        sl = slice(512 * c4, 512 * (c4 + 1))
        nc.sync.dma_start(afT_bf[:, sl], afT[:, sl])

    # ---------------- projection (full, for pb) ----------------
    projT_bf = const.tile([32, N], bf16)
    for c4 in range(4):
        pt = psA.tile([32, 512], f32, name=f"prj{c4}", tag="prj")
        nc.tensor.matmul(pt[:], WpT_bf[:], afT_bf[:, 512 * c4 : 512 * (c4 + 1)],
                         start=True, stop=True)
        nc.vector.tensor_copy(projT_bf[:, 512 * c4 : 512 * (c4 + 1)], pt[:])

    # pbT replicated 4x in partitions: lhsT = WbT4 [32, 128]
    pbT_rep4 = const.tile([128, N], bf16)
    for c4 in range(4):
        pt = psA.tile([128, 512], f32, name=f"pb{c4}", tag="pb")
        nc.tensor.matmul(pt[:], WbT4_bf[:], projT_bf[:, 512 * c4 : 512 * (c4 + 1)],
                         start=True, stop=True)
        nc.scalar.activation(pbT_rep4[:, 512 * c4 : 512 * (c4 + 1)], pt[:], Act.Copy)

    # ---------------- slab projection (for pa) ----------------
    projTs_aug = const.tile([33, ROWS], bf16)
    pts = psA.tile([32, ROWS], f32, name="prjs", tag="prj")
    nc.tensor.matmul(pts[:], WpT_bf[:], afs_bf[:], start=True, stop=True)
    nc.vector.tensor_copy(projTs_aug[:32, :], pts[:])
    nc.gpsimd.memset(projTs_aug[32:33, :], 1.0)

    # paT (+b1) replicated 4x in partitions: [128, ROWS] f32
    paT_sb = const.tile([128, ROWS], f32)
    pap = psA.tile([128, ROWS], f32, name="pap", tag="pap")
    nc.tensor.matmul(pap[:], WaTb_bf[:], projTs_aug[:], start=True, stop=True)
    nc.vector.tensor_copy(paT_sb[:], pap[:])

    # c_cols [128, NST*4]: column q = ST*4 + c holds pa(+b1) for the 4 i's
    # (R, a) at partition blocks b = 2R + a; i_local = ST*16 + R*8 + 2c + a.
    c_cols = const.tile([128, NST * 4], f32)
    paT_v = paT_sb[:].rearrange("p (st ii) -> p st ii", ii=16)
    cc_v = c_cols[:].rearrange("p (st c) -> p st c", c=4)
    for b in range(4):
        R, a = b // 2, b % 2
        src = paT_v[32 * b : 32 * b + 32, :, R * 8 + a : R * 8 + 8 : 2]
        dst = cc_v[32 * b : 32 * b + 32, :, :]
        nc.vector.tensor_copy(dst, src)

    # ---------------- main loop ----------------
    pre_ctx.close()  # release preamble PSUM pool
    psB = ctx.enter_context(tc.tile_pool(name="psB", bufs=3, space="PSUM"))
    psL = ctx.enter_context(tc.tile_pool(name="psL", bufs=2, space="PSUM"))
    # logits psum bank: 8 units of 64 slot-cols; unit u = (ST, jt) mod 8
    # Software pipeline: L3 + sigmoid + DMA run one unit behind L2/B.
    state = {"logits_ps": None, "sig_sb": None}
    pending = []  # (u_abs, hr) awaiting L3

    def do_L3(u_abs, hr):
        u = u_abs % 8
        if u == 0:
            state["logits_ps"] = psL.tile([128, 512], f32, name=f"lg{u_abs}", tag="lg")
            state["sig_sb"] = sigp.tile([128, 512], f32, name=f"sg{u_abs}", tag="sg")
        logits_ps, sig_sb = state["logits_ps"], state["sig_sb"]
        # slot layout within unit: s*16 + R*8 + m  (i = ST*16 + R*8 + m)
        for R in range(2):
            for s in range(4):
                off = u * 64 + s * 16 + R * 8
                nc.tensor.matmul(
                    logits_ps[:, off : off + 8],
                    hr[:, 512 * R + 128 * s : 512 * R + 128 * (s + 1)],
                    W3bd_bf[:],
                    start=True,
                    stop=True,
                )
        if u == 7:
            # C: sigmoid over the full bank, then one DMA per super-tile
            # half (jt and s merge into a single 128-row block dim t).
            nc.scalar.activation(sig_sb[:], logits_ps[:], Act.Sigmoid, bias=b3c[:])
            for STh in range(2):
                ST2 = (u_abs - 7 + STh * NJT) // NJT
                sb_v = sig_sb[:, STh * 256 : (STh + 1) * 256].rearrange(
                    "jp (t i) -> jp t i", t=16
                )
                dr_v = out_d[:, ST2 * 16 : (ST2 + 1) * 16].rearrange(
                    "(t jp) i -> jp t i", t=16
                )
                (nc.sync if STh == 0 else nc.scalar).dma_start(dr_v, sb_v)

    def issue_A(ST, h1_ST, c):
        nc.vector.tensor_scalar(
            h1_ST[:, N * c : N * (c + 1)],
            pbT_rep4[:],
            c_cols[:, ST * 4 + c : ST * 4 + c + 1],
            0.0,
            Alu.add,
            Alu.max,
        )

    # h1 for ST 0 up front; h1 for ST+1 is issued spread across ST's units.
    h1_tiles = {0: h1p.tile([128, 4 * N], bf16, name="h1_0", tag="h1")}
    for c in range(4):
        issue_A(0, h1_tiles[0], c)

    for ST in range(NST):
        h1_ST = h1_tiles.pop(ST)
        for jt in range(NJT):
            u_abs = ST * NJT + jt
            if ST + 1 < NST:
                if jt == 0:
                    h1_tiles[ST + 1] = h1p.tile(
                        [128, 4 * N], bf16, name=f"h1_{ST + 1}", tag="h1"
                    )
                issue_A(ST + 1, h1_tiles[ST + 1], jt)

            # L2: 8 packed matmuls -> one psum tile [128, 1024] (R at col 512R)
            ps = psB.tile([128, 1024], f32, name=f"l2_{u_abs}", tag="l2")
            for R in range(2):
                for c in range(4):
                    nc.tensor.matmul(
                        ps[32 * c : 32 * c + 32, 512 * R : 512 * (R + 1)],
                        W2bd_bf[64 * R : 64 * R + 64, :],
                        h1_ST[64 * R : 64 * R + 64, N * c + 512 * jt : N * c + 512 * (jt + 1)],
                        start=True,
                        stop=True,
                        tile_position=(64 * R, 32 * c),
                    )
            # B-pass: relu(psum + b2) -> bf16; one instr per unit, 3/4 on ACT
            hr = h2p.tile([128, 1024], bf16, name=f"h2r_{u_abs}", tag="h2r")
            if u_abs % 4 != 3:
                nc.scalar.activation(hr[:], ps[:], Act.Relu, bias=b2c[:])
            else:
                nc.vector.tensor_scalar(hr[:], ps[:], b2c[:], 0.0, Alu.add, Alu.max)

            pending.append((u_abs, hr))
            if len(pending) > 2:
                do_L3(*pending.pop(0))
    while pending:
        do_L3(*pending.pop(0))
    ctx.close()


def _host_inputs(asset_features, Wp, bp, W1, b1, W2, b2, W3, b3, core):
    af = np.asarray(asset_features, np.float32)
    ones_n = np.ones((1, N), np.float32)
    ones_r = np.ones((1, ROWS), np.float32)
    sl = slice(core * ROWS, (core + 1) * ROWS)

    Wa = W1[:, :32]  # [32k, 32f]
    Wb = W1[:, 32:]

    WaTb_base = np.concatenate([Wa.T, b1[None, :]], axis=0)          # [33, 32]
    WbT4 = np.tile(Wb.T, (1, 4)).astype(np.float32)                  # [32, 128]
    WaTb = np.tile(WaTb_base, (1, 4)).astype(np.float32)             # [33, 128]

    # L2 block-diag [64, 32]: rows 32a+k, cols 16a+h = W2[h, k]
    W2bd64 = np.zeros((64, 32), np.float32)
    for a in range(2):
        W2bd64[32 * a : 32 * a + 32, 16 * a : 16 * a + 16] = W2.T    # [k, h]
    W2bd = np.tile(W2bd64, (2, 1))                                   # [128, 32]

    # L3 block-diag [128, 8]: rows 32c+16a+h, col m = 2c+a -> w3[h]
    W3bd = np.zeros((128, 8), np.float32)
    for c in range(4):
        for a in range(2):
            W3bd[32 * c + 16 * a : 32 * c + 16 * a + 16, 2 * c + a] = W3[0]

    b2col = np.tile(b2, 8).reshape(128, 1).astype(np.float32)
    b3col = np.full((128, 1), b3[0], np.float32)

    return {
        "afT": np.ascontiguousarray(
            np.concatenate([af.T, ones_n], axis=0).astype(ml_dtypes.bfloat16)
        ),
        "afs": np.ascontiguousarray(
            np.concatenate([af.T[:, sl], ones_r], axis=0).astype(ml_dtypes.bfloat16)
        ),
        "WpT": np.concatenate([Wp.T, bp[None, :]], axis=0).astype(ml_dtypes.bfloat16),
        "WaTb": WaTb.astype(ml_dtypes.bfloat16),
        "WbT4": WbT4.astype(ml_dtypes.bfloat16),
        "W2bd": W2bd.astype(ml_dtypes.bfloat16),
        "W3bd": W3bd.astype(ml_dtypes.bfloat16),
        "b2col": b2col,
        "b3col": b3col,
    }


def kernel(asset_features, Wp, bp, W1, b1, W2, b2, W3, b3, _trace=False):
    if "nc" not in _CACHE:
        _CACHE["nc"] = _build_program()
    nc = _CACHE["nc"]

    in_maps = [
        _host_inputs(asset_features, Wp, bp, W1, b1, W2, b2, W3, b3, core)
        for core in range(NCORES)
    ]
    res = run_bass_kernel_spmd(nc, in_maps, list(range(NCORES)), trace=_trace)
    _CACHE["last_exec_time_ns"] = res.exec_time_ns

    out = np.empty((N, N), np.float32)
    for c in range(NCORES):
        out[c * ROWS : (c + 1) * ROWS, :] = res.results[c]["outT"].T
    np.fill_diagonal(out, 1.0)
    return out
